# revision 13
# baseline (speedup 1.0000x reference)
# BatchChildSumTreeLSTM on 8 Trainium2 NeuronCores (Bass/Tile).
#
# Strategy: data-parallel over the 1024 trees (128 per core); weights and the
# 50000x300 embedding table are replicated per core. Inside each core:
#   - embedding rows are fetched with indirect (gather) DMA, 128 rows per call
#     in "quad" layout: partition p = (node_pair_idx p//64, tree p%64)
#   - gathered [row, dim] tiles are PE-transposed to [dim, row] so all gate
#     matmuls keep the contraction dim on partitions (K tiles of 128/128/44)
#   - levels are processed bottom-up; only pairwise child sums (sum h, sum f*c)
#     cross levels.  own_c/own_h are zero in the reference (nodes are
#     processed before their own state is written), so f = sigmoid(x@W_fx +
#     b_fx + b_fh) and c = i*u + sum_children(f_j*c_j); W_fh drops out.
#   - matmul operands are bf16 (fp32 accumulate in PSUM); end-to-end logp
#     error vs the fp32 reference is ~4e-4 absolute on a ~1.75 scale.
# Outputs: per-core logp [128,5] and the per-core sum of logp[gold]; the host
# concatenates logp shards and finishes loss = -sum/1024.

import numpy as np
from contextlib import ExitStack

import ml_dtypes

import concourse.bacc as bacc
import concourse.bass as bass
import concourse.mybir as mybir
import concourse.tile as tile
from concourse.bass import IndirectOffsetOnAxis
from concourse.bass_utils import run_bass_kernel_spmd
from concourse.masks import make_identity

DT = mybir.dt
F32 = DT.float32
BF16 = DT.bfloat16

V, DIN, H, L = 50000, 300, 300, 5
B, DEPTH, NPT = 1024, 6, 127
NCORES, BSH = 8, 128
G = 64                    # trees per pass
NPASS = BSH // G          # 2
NQ = 64                   # gather quads per pass (63 node pairs + root dup)
KT = 3                    # dim tiles over 300: sizes 128/128/44
TSZ = [128, 128, 44]
TOFF = [0, 128, 256]
AF = mybir.ActivationFunctionType

# gate order in w4/b4: i, f, o, u ; wh columns: i, o, u
H_COL = {0: 0, 2: 300, 3: 600}
FUNC = {0: AF.Sigmoid, 1: AF.Sigmoid, 2: AF.Sigmoid, 3: AF.Tanh}


def quad_list():
    """Node pairs in bottom-up processing order; 63 pairs + (0,0) for roots."""
    quads = []
    for lvl in range(DEPTH, 0, -1):
        s, e = 2 ** lvl - 1, 2 ** (lvl + 1) - 1
        nodes = list(range(s, e))
        for i in range(0, len(nodes), 2):
            quads.append((nodes[i], nodes[i + 1]))
    quads.append((0, 0))
    return quads


def build_program():
    nc = bacc.Bacc(trn_type="TRN2", target_bir_lowering=False, debug=False)

    gidx_d = nc.dram_tensor("gidx", [128, NPASS * NQ], DT.int32, kind="ExternalInput").ap()
    emb_d = nc.dram_tensor("emb", [V, DIN], F32, kind="ExternalInput").ap()
    w4_d = nc.dram_tensor("w4", [KT, 128, 1200], BF16, kind="ExternalInput").ap()
    wh_d = nc.dram_tensor("wh", [KT, 128, 900], BF16, kind="ExternalInput").ap()
    b4_d = nc.dram_tensor("b4", [128, 12], F32, kind="ExternalInput").ap()
    wout_d = nc.dram_tensor("wout", [KT, 128, L], BF16, kind="ExternalInput").ap()
    bout_d = nc.dram_tensor("bout", [L, 1], F32, kind="ExternalInput").ap()
    oneh_d = nc.dram_tensor("onehot", [BSH, L], F32, kind="ExternalInput").ap()
    logp_d = nc.dram_tensor("logp", [BSH, L], F32, kind="ExternalOutput").ap()
    nll_d = nc.dram_tensor("nll", [1, 1], F32, kind="ExternalOutput").ap()

    with ExitStack() as ctx:
        tc = ctx.enter_context(tile.TileContext(nc))
        singles = ctx.enter_context(tc.tile_pool(name="singles", bufs=1))
        graw_p = ctx.enter_context(tc.tile_pool(name="grawp", bufs=8))
        xt_p = ctx.enter_context(tc.tile_pool(name="xtp", bufs=4))
        gate_p = ctx.enter_context(tc.tile_pool(name="gatep", bufs=2))
        state_p = ctx.enter_context(tc.tile_pool(name="statep", bufs=1))
        ps_g = ctx.enter_context(tc.tile_pool(name="psg", bufs=3, space="PSUM"))
        ps_t = ctx.enter_context(tc.tile_pool(name="pst", bufs=2, space="PSUM"))

        ident = singles.tile([128, 128], F32, name="ident")
        make_identity(nc, ident)
        w4_sb = singles.tile([128, KT, 1200], BF16, name="w4sb")
        nc.sync.dma_start(out=w4_sb, in_=w4_d.rearrange("k p c -> p k c"))
        wh_sb = singles.tile([128, KT, 900], BF16, name="whsb")
        nc.sync.dma_start(out=wh_sb, in_=wh_d.rearrange("k p c -> p k c"))
        wout_sb = singles.tile([128, KT, L], BF16, name="woutsb")
        nc.sync.dma_start(out=wout_sb, in_=wout_d.rearrange("k p c -> p k c"))
        b4_sb = singles.tile([128, 12], F32, name="b4sb")
        nc.sync.dma_start(out=b4_sb, in_=b4_d)
        bout_sb = singles.tile([L, 1], F32, name="boutsb")
        nc.sync.dma_start(out=bout_sb, in_=bout_d)
        oneh_sb = singles.tile([BSH, L], F32, name="onehsb")
        nc.sync.dma_start(out=oneh_sb, in_=oneh_d)
        gidx_sb = singles.tile([128, NPASS * NQ], DT.int32, name="gidxsb")
        nc.sync.dma_start(out=gidx_sb, in_=gidx_d)
        ones_sb = singles.tile([BSH, 1], F32, name="onessb")
        nc.vector.memset(ones_sb, 1.0)

        ident5 = singles.tile([L, L], F32, name="ident5")
        make_identity(nc, ident5)
        h_root = state_p.tile([128, KT, BSH], BF16, name="hroot", tag="hroot")

        for g in range(NPASS):
            qbase = 0
            ch_in = cf_in = None
            for lvl in range(DEPTH, -1, -1):
                nl = 2 ** lvl
                rows = nl * G if lvl > 0 else G
                nq_lvl = max(nl // 2, 1)
                if lvl > 0:
                    par = "A" if lvl % 2 == 0 else "B"
                    prow = (nl // 2) * G
                    ch_out = state_p.tile([128, KT, prow], BF16, name=f"ch_{g}_{lvl}", tag="ch" + par)
                    cf_out = state_p.tile([128, KT, prow], F32, name=f"cf_{g}_{lvl}", tag="cf" + par)
                else:
                    ch_out = cf_out = None
                SC = 1024
                nsc = max(rows // SC, 1)
                scN = min(rows, SC)          # rows per superchunk
                qpsc = nq_lvl // nsc         # quads per superchunk
                for c in range(nsc):
                    q0 = g * NQ + qbase + c * qpsc
                    xt = xt_p.tile([128, KT, SC], BF16, name="xt", tag="xt")
                    ncols = 128 if lvl > 0 else 64
                    graw = None
                    for qi in range(qpsc):
                        if qi % 4 == 0:
                            graw = graw_p.tile([128, 4, DIN], F32, name="graw", tag="graw")
                            for qj in range(qi, min(qi + 4, qpsc)):
                                # one gather per quad: [128,1] index AP is the
                                # only form the HW descriptor unroll handles
                                nc.gpsimd.indirect_dma_start(
                                    out=graw[:, qj - qi, :], out_offset=None,
                                    in_=emb_d,
                                    in_offset=IndirectOffsetOnAxis(
                                        ap=gidx_sb[:, q0 + qj:q0 + qj + 1], axis=0),
                                )
                        pst = ps_t.tile([128, KT * 128], F32, name="pst", tag="pst")
                        for k in range(KT):
                            nc.tensor.transpose(
                                out=pst[:TSZ[k], k * 128:(k + 1) * 128],
                                in_=graw[:, qi % 4, TOFF[k]:TOFF[k] + TSZ[k]],
                                identity=ident)
                        # fp32->bf16 casts: k0+k1 in one op, the 44-row k2 alone
                        nc.vector.tensor_copy(
                            out=xt[:, 0:2, qi * 128: qi * 128 + ncols],
                            in_=pst[:, 0:256].rearrange("p (k c) -> p k c", k=2)[:, :, :ncols])
                        nc.vector.tensor_copy(
                            out=xt[:TSZ[2], 2, qi * 128: qi * 128 + ncols],
                            in_=pst[:TSZ[2], 256:256 + ncols])
                    c0 = c * SC
                    N = scN
                    for m in range(KT):
                        msz, moff = TSZ[m], TOFF[m]
                        gate_sb = {}
                        for gi in range(4):
                            if lvl == 0 and gi == 1:
                                continue
                            ps = ps_g.tile([128, SC], F32, name="psgate", tag="psg")
                            has_h = (lvl < DEPTH) and (gi in H_COL)
                            for mc in range(0, N, 512):
                                mN = min(512, N - mc)
                                for k in range(KT):
                                    nc.tensor.matmul(
                                        out=ps[:msz, mc:mc + mN],
                                        lhsT=w4_sb[:TSZ[k], k, gi * 300 + moff: gi * 300 + moff + msz],
                                        rhs=xt[:TSZ[k], k, mc:mc + mN],
                                        start=(k == 0), stop=(k == KT - 1 and not has_h))
                                if has_h:
                                    for k in range(KT):
                                        nc.tensor.matmul(
                                            out=ps[:msz, mc:mc + mN],
                                            lhsT=wh_sb[:TSZ[k], k, H_COL[gi] + moff: H_COL[gi] + moff + msz],
                                            rhs=ch_in[:TSZ[k], k, c0 + mc:c0 + mc + mN],
                                            start=False, stop=(k == KT - 1))
                            gsb = gate_p.tile([128, SC], F32, name=f"g{gi}", tag=f"g{gi}")
                            nc.scalar.activation(
                                out=gsb[:msz, :N], in_=ps[:msz, :N], func=FUNC[gi],
                                bias=b4_sb[:msz, gi * 3 + m: gi * 3 + m + 1])
                            gate_sb[gi] = gsb
                        c_sb = gate_p.tile([128, SC], F32, name="csb", tag="csb")
                        nc.vector.tensor_mul(c_sb[:msz, :N], gate_sb[0][:msz, :N], gate_sb[3][:msz, :N])
                        if lvl < DEPTH:
                            nc.vector.tensor_add(c_sb[:msz, :N], c_sb[:msz, :N], cf_in[:msz, m, c0:c0 + N])
                        tc_sb = gate_p.tile([128, SC], F32, name="tcsb", tag="tcsb")
                        nc.scalar.activation(out=tc_sb[:msz, :N], in_=c_sb[:msz, :N], func=AF.Tanh)
                        if lvl > 0:
                            h_sb = gate_p.tile([128, SC], F32, name="hsb", tag="hsb")
                            nc.vector.tensor_mul(h_sb[:msz, :N], gate_sb[2][:msz, :N], tc_sb[:msz, :N])
                            fc_sb = gate_p.tile([128, SC], F32, name="fcsb", tag="fcsb")
                            nc.vector.tensor_mul(fc_sb[:msz, :N], gate_sb[1][:msz, :N], c_sb[:msz, :N])
                            hv = h_sb[:msz, :N].rearrange("p (nb two g) -> p nb two g", two=2, g=G)
                            fv = fc_sb[:msz, :N].rearrange("p (nb two g) -> p nb two g", two=2, g=G)
                            po = c0 // 2
                            cho = ch_out[:msz, m, po:po + N // 2].rearrange("p (nb g) -> p nb g", g=G)
                            cfo = cf_out[:msz, m, po:po + N // 2].rearrange("p (nb g) -> p nb g", g=G)
                            nc.vector.tensor_add(cho, hv[:, :, 0, :], hv[:, :, 1, :])
                            nc.vector.tensor_add(cfo, fv[:, :, 0, :], fv[:, :, 1, :])
                        else:
                            nc.vector.tensor_mul(
                                h_root[:msz, m, g * G:(g + 1) * G],
                                gate_sb[2][:msz, :G], tc_sb[:msz, :G])
                qbase += nq_lvl
                ch_in, cf_in = ch_out, cf_out

        # classifier + log_softmax + sum(logp[gold]) over all 128 trees
        ps_cls = ps_g.tile([L, BSH], F32, name="pscls", tag="psg")
        for k in range(KT):
            nc.tensor.matmul(out=ps_cls, lhsT=wout_sb[:TSZ[k], k, :],
                             rhs=h_root[:TSZ[k], k, :],
                             start=(k == 0), stop=(k == KT - 1))
        logitsT = gate_p.tile([L, BSH], F32, name="logitsT", tag="mA")
        nc.vector.tensor_scalar_add(logitsT, ps_cls, bout_sb[:, 0:1])
        ps_lg = ps_g.tile([BSH, L], F32, name="pslg", tag="psg")
        nc.tensor.transpose(out=ps_lg, in_=logitsT, identity=ident5)
        logits = gate_p.tile([BSH, L], F32, name="logits", tag="mB")
        nc.vector.tensor_copy(logits, ps_lg)
        msb = gate_p.tile([BSH, 1], F32, name="msb", tag="m1")
        nc.vector.reduce_max(out=msb, in_=logits, axis=mybir.AxisListType.X)
        negm = gate_p.tile([BSH, 1], F32, name="negm", tag="m2")
        nc.vector.tensor_scalar_mul(negm, msb, -1.0)
        esb = gate_p.tile([BSH, L], F32, name="esb", tag="mC")
        nc.scalar.activation(out=esb, in_=logits, func=AF.Exp, bias=negm[:, 0:1])
        ssb = gate_p.tile([BSH, 1], F32, name="ssb", tag="m3")
        nc.vector.reduce_sum(out=ssb, in_=esb, axis=mybir.AxisListType.X)
        lssb = gate_p.tile([BSH, 1], F32, name="lssb", tag="m4")
        nc.scalar.activation(out=lssb, in_=ssb, func=AF.Ln)
        tot = gate_p.tile([BSH, 1], F32, name="tot", tag="m5")
        nc.vector.tensor_add(tot, msb, lssb)
        logp_sb = gate_p.tile([BSH, L], F32, name="logpsb", tag="mD")
        nc.vector.tensor_scalar_sub(logp_sb, logits, tot[:, 0:1])
        nc.sync.dma_start(out=logp_d, in_=logp_sb)
        prod = gate_p.tile([BSH, L], F32, name="prod", tag="mE")
        nc.vector.tensor_mul(prod, logp_sb, oneh_sb)
        ptree = gate_p.tile([BSH, 1], F32, name="ptree", tag="m6")
        nc.vector.reduce_sum(out=ptree, in_=prod, axis=mybir.AxisListType.X)
        ps_nll = ps_g.tile([1, 1], F32, name="psnll", tag="psg")
        nc.tensor.matmul(out=ps_nll, lhsT=ptree, rhs=ones_sb, start=True, stop=True)
        nll_sb = gate_p.tile([1, 1], F32, name="nllsb", tag="m7")
        nc.vector.tensor_copy(nll_sb, ps_nll)
        nc.sync.dma_start(out=nll_d, in_=nll_sb)
    nc.finalize()
    return nc


def _pad_ktiles(w, cols):
    """[300, cols] fp32 -> [KT, 128, cols] bf16 (zero-padded rows 300..383)."""
    out = np.zeros((KT, 128, cols), dtype=ml_dtypes.bfloat16)
    for k in range(KT):
        out[k, :TSZ[k], :] = w[TOFF[k]:TOFF[k] + TSZ[k], :].astype(ml_dtypes.bfloat16)
    return out


def prep_in_maps(inputs):
    wi = np.asarray(inputs["word_idx"]).astype(np.int32)
    gold = np.asarray(inputs["gold"]).astype(np.int64)
    emb = np.ascontiguousarray(np.asarray(inputs["embedding"], dtype=np.float32))
    W4 = np.concatenate(
        [np.asarray(inputs["W_" + n], dtype=np.float32) for n in ("ix", "fx", "ox", "ux")],
        axis=1)
    Wh = np.concatenate(
        [np.asarray(inputs["W_" + n], dtype=np.float32) for n in ("ih", "oh", "uh")],
        axis=1)
    b4 = np.concatenate([
        np.asarray(inputs["b_ix"]) + np.asarray(inputs["b_ih"]),
        np.asarray(inputs["b_fx"]) + np.asarray(inputs["b_fh"]),
        np.asarray(inputs["b_ox"]) + np.asarray(inputs["b_oh"]),
        np.asarray(inputs["b_ux"]) + np.asarray(inputs["b_uh"]),
    ]).astype(np.float32)
    b4_mat = np.zeros((128, 12), np.float32)
    for gi in range(4):
        for m in range(KT):
            b4_mat[:TSZ[m], gi * 3 + m] = b4[gi * 300 + TOFF[m]: gi * 300 + TOFF[m] + TSZ[m]]
    wout = np.asarray(inputs["W_out"], dtype=np.float32)
    bout = np.ascontiguousarray(
        np.asarray(inputs["b_out"], dtype=np.float32).reshape(L, 1))
    quads = quad_list()
    qa = np.array([q[0] for q in quads])
    qb = np.array([q[1] for q in quads])
    eye = np.eye(L, dtype=np.float32)

    w4_p = _pad_ktiles(W4, 1200)
    wh_p = _pad_ktiles(Wh, 900)
    wout_p = _pad_ktiles(wout, L)

    in_maps = []
    for c in range(NCORES):
        t0 = c * BSH
        gidx = np.empty((128, NPASS * NQ), np.int32)
        for g in range(NPASS):
            blk = wi[t0 + g * G: t0 + (g + 1) * G]               # [64, 127]
            gidx[0:G, g * NQ:(g + 1) * NQ] = blk[:, qa]
            gidx[G:2 * G, g * NQ:(g + 1) * NQ] = blk[:, qb]
        in_maps.append(dict(
            gidx=np.ascontiguousarray(gidx), emb=emb, w4=w4_p, wh=wh_p,
            b4=b4_mat, wout=wout_p, bout=bout,
            onehot=np.ascontiguousarray(eye[gold[t0:t0 + BSH]])))
    return in_maps


_PROG = None


def _get_prog():
    global _PROG
    if _PROG is None:
        _PROG = build_program()
    return _PROG


def _assemble(results):
    logp = np.concatenate([results[c]["logp"] for c in range(NCORES)], axis=0)
    tot = sum(float(results[c]["nll"][0, 0]) for c in range(NCORES))
    loss = np.float32(-tot / B)
    return np.ascontiguousarray(logp.astype(np.float32)), loss


def kernel(**inputs):
    nc = _get_prog()
    in_maps = prep_in_maps(inputs)
    res = run_bass_kernel_spmd(nc, in_maps, list(range(NCORES)))
    return _assemble(res.results)


def kernel_profiled(**inputs):
    """Same as kernel() but with NTFF tracing; returns (outputs, exec_time_ns)."""
    nc = _get_prog()
    in_maps = prep_in_maps(inputs)
    res = run_bass_kernel_spmd(nc, in_maps, list(range(NCORES)), trace=True)
    return _assemble(res.results), res.exec_time_ns


# revision 15
# speedup vs baseline: 1.4415x; 1.4415x over previous
# BatchChildSumTreeLSTM on 8 Trainium2 NeuronCores (Bass/Tile).
#
# Strategy: data-parallel over the 1024 trees (128 per core); weights and the
# 50000x300 embedding table are replicated per core. Inside each core:
#   - embedding rows are fetched with indirect (gather) DMA, 128 rows per call
#     in "quad" layout: partition p = (node_pair_idx p//64, tree p%64)
#   - gathered [row, dim] tiles are PE-transposed to [dim, row] so all gate
#     matmuls keep the contraction dim on partitions (3 K-tiles of 100)
#   - the three 100-dim M-tiles of each gate land in the free dimension of one
#     3-bank PSUM tile [100, 3, 512], so each gate takes a single activation op
#   - gate biases ride the x-side matmul for free: the first K-tile carries an
#     extra ones-row in the moving operand and the bias row in the weights
#   - levels are processed bottom-up; only pairwise child sums (sum h, sum f*c)
#     cross levels.  own_c/own_h are zero in the reference (nodes are
#     processed before their own state is written), so f = sigmoid(x@W_fx +
#     b_fx + b_fh) and c = i*u + sum_children(f_j*c_j); W_fh drops out.
#   - matmul operands are bf16 (fp32 accumulate in PSUM); end-to-end logp
#     error vs the fp32 reference is ~4e-4 absolute on a ~1.75 scale.
# Outputs: per-core logp [128,5] and the per-core sum of logp[gold]; the host
# concatenates logp shards and finishes loss = -sum/1024.

import numpy as np
from contextlib import ExitStack

import ml_dtypes

import concourse.bacc as bacc
import concourse.bass as bass
import concourse.mybir as mybir
import concourse.tile as tile
from concourse.bass import IndirectOffsetOnAxis
from concourse.bass_utils import run_bass_kernel_spmd
from concourse.masks import make_identity

DT = mybir.dt
F32 = DT.float32
BF16 = DT.bfloat16

V, DIN, H, L = 50000, 300, 300, 5
B, DEPTH, NPT = 1024, 6, 127
NCORES, BSH = 8, 128
G = 64                    # trees per pass
NPASS = BSH // G          # 2
NQ = 64                   # gather quads per pass (63 node pairs + root dup)
KT = 3                    # dim tiles over 300: 3 x 100
TS = 100
AF = mybir.ActivationFunctionType

# gate order in w4: i, f, o, u ; wh columns: i, o, u
H_COL = {0: 0, 2: 300, 3: 600}
FUNC = {0: AF.Sigmoid, 1: AF.Sigmoid, 2: AF.Sigmoid, 3: AF.Tanh}
# sbuf slot-tag reuse pairs: (i,tc) (f,fc) (o,h) (u,c)
GTAG = {0: "g0", 1: "g1", 2: "g2", 3: "g3"}


def quad_list():
    """Node pairs in bottom-up processing order; 63 pairs + (0,0) for roots."""
    quads = []
    for lvl in range(DEPTH, 0, -1):
        s, e = 2 ** lvl - 1, 2 ** (lvl + 1) - 1
        nodes = list(range(s, e))
        for i in range(0, len(nodes), 2):
            quads.append((nodes[i], nodes[i + 1]))
    quads.append((0, 0))
    return quads


def build_program():
    nc = bacc.Bacc(trn_type="TRN2", target_bir_lowering=False, debug=False)

    gidx_d = nc.dram_tensor("gidx", [128, NPASS * NQ], DT.int32, kind="ExternalInput").ap()
    emb_d = nc.dram_tensor("emb", [V, DIN], F32, kind="ExternalInput").ap()
    w4_d = nc.dram_tensor("w4", [KT, 128, 1200], BF16, kind="ExternalInput").ap()
    wh_d = nc.dram_tensor("wh", [KT, 128, 900], BF16, kind="ExternalInput").ap()
    wout_d = nc.dram_tensor("wout", [KT, 128, L], BF16, kind="ExternalInput").ap()
    oneh_d = nc.dram_tensor("onehot", [BSH, L], F32, kind="ExternalInput").ap()
    logp_d = nc.dram_tensor("logp", [BSH, L], F32, kind="ExternalOutput").ap()
    nll_d = nc.dram_tensor("nll", [1, 1], F32, kind="ExternalOutput").ap()

    with ExitStack() as ctx:
        tc = ctx.enter_context(tile.TileContext(nc))
        singles = ctx.enter_context(tc.tile_pool(name="singles", bufs=1))
        graw_p = ctx.enter_context(tc.tile_pool(name="grawp", bufs=8))
        xt_p = ctx.enter_context(tc.tile_pool(name="xtp", bufs=4))
        gate_p = ctx.enter_context(tc.tile_pool(name="gatep", bufs=2))
        state_p = ctx.enter_context(tc.tile_pool(name="statep", bufs=1))
        ps_g = ctx.enter_context(tc.tile_pool(name="psg", bufs=2, space="PSUM"))
        ps_t = ctx.enter_context(tc.tile_pool(name="pst", bufs=2, space="PSUM"))

        ident = singles.tile([128, 128], F32, name="ident")
        make_identity(nc, ident)
        ident5 = singles.tile([L, L], F32, name="ident5")
        make_identity(nc, ident5)
        w4_sb = singles.tile([128, KT, 1200], BF16, name="w4sb")
        nc.sync.dma_start(out=w4_sb, in_=w4_d.rearrange("k p c -> p k c"))
        wh_sb = singles.tile([128, KT, 900], BF16, name="whsb")
        nc.sync.dma_start(out=wh_sb, in_=wh_d.rearrange("k p c -> p k c"))
        wout_sb = singles.tile([128, KT, L], BF16, name="woutsb")
        nc.sync.dma_start(out=wout_sb, in_=wout_d.rearrange("k p c -> p k c"))
        oneh_sb = singles.tile([BSH, L], F32, name="onehsb")
        nc.sync.dma_start(out=oneh_sb, in_=oneh_d)
        gidx_sb = singles.tile([128, NPASS * NQ], DT.int32, name="gidxsb")
        nc.sync.dma_start(out=gidx_sb, in_=gidx_d)
        ones_sb = singles.tile([BSH, 1], F32, name="onessb")
        nc.vector.memset(ones_sb, 1.0)

        # h_root[100, 0, :] is a ones-row so the classifier bias rides the matmul
        h_root = state_p.tile([128, KT, BSH], BF16, name="hroot", tag="hroot")
        nc.vector.memset(h_root[96:128, 0, :], 1.0)  # rows 96:100 later overwritten by h

        for g in range(NPASS):
            qbase = 0
            ch_in = cf_in = None
            for lvl in range(DEPTH, -1, -1):
                nl = 2 ** lvl
                rows = nl * G if lvl > 0 else G
                nq_lvl = max(nl // 2, 1)
                if lvl > 0:
                    par = "A" if lvl % 2 == 0 else "B"
                    prow = (nl // 2) * G
                    ch_out = state_p.tile([TS, KT, prow], BF16, name=f"ch_{g}_{lvl}", tag="ch" + par)
                    cf_out = state_p.tile([TS, KT, prow], F32, name=f"cf_{g}_{lvl}", tag="cf" + par)
                else:
                    ch_out = cf_out = None
                nchunk = max(rows // 512, 1)
                qpc = nq_lvl // nchunk
                N = min(rows, 512)
                for c in range(nchunk):
                    q0 = g * NQ + qbase + c * qpc
                    graw = graw_p.tile([128, 4, DIN], F32, name="graw", tag="graw")
                    for qi in range(qpc):
                        # one gather per quad: [128,1] index AP is the only
                        # form the HW descriptor unroll handles correctly
                        nc.gpsimd.indirect_dma_start(
                            out=graw[:, qi, :], out_offset=None,
                            in_=emb_d,
                            in_offset=IndirectOffsetOnAxis(
                                ap=gidx_sb[:, q0 + qi:q0 + qi + 1], axis=0),
                        )
                    xt = xt_p.tile([128, KT, 512], BF16, name="xt", tag="xt")
                    nc.vector.memset(xt[96:128, 0, :], 1.0)  # ones-row at 100; 96:100 overwritten by cast
                    ncols = 128 if lvl > 0 else 64
                    for qi in range(qpc):
                        pst = ps_t.tile([TS, KT * 128], F32, name="pst", tag="pst")
                        for k in range(KT):
                            nc.tensor.transpose(
                                out=pst[:, k * 128:(k + 1) * 128],
                                in_=graw[:, qi, k * TS:(k + 1) * TS],
                                identity=ident)
                        # single fp32->bf16 cast per quad (3 K-tiles at once)
                        nc.vector.tensor_copy(
                            out=xt[:TS, :, qi * 128: qi * 128 + ncols],
                            in_=pst.rearrange("p (k c) -> p k c", k=KT)[:, :, :ncols])
                    c0 = c * 512
                    gate_sb = {}
                    for gi in range(4):
                        if lvl == 0 and gi == 1:
                            continue
                        ps = ps_g.tile([TS, KT, 512], F32, name="psgate", tag="psg")
                        has_h = (lvl < DEPTH) and (gi in H_COL)
                        for m in range(KT):
                            for k in range(KT):
                                ksz = TS + 1 if k == 0 else TS  # ones/bias row
                                nc.tensor.matmul(
                                    out=ps[:, m, :N],
                                    lhsT=w4_sb[:ksz, k, gi * 300 + m * TS: gi * 300 + (m + 1) * TS],
                                    rhs=xt[:ksz, k, :N],
                                    start=(k == 0), stop=(k == KT - 1 and not has_h))
                            if has_h:
                                for k in range(KT):
                                    nc.tensor.matmul(
                                        out=ps[:, m, :N],
                                        lhsT=wh_sb[:TS, k, H_COL[gi] + m * TS: H_COL[gi] + (m + 1) * TS],
                                        rhs=ch_in[:, k, c0:c0 + N],
                                        start=False, stop=(k == KT - 1))
                        gsb = gate_p.tile([TS, KT, 512], F32, name=f"g{gi}", tag=GTAG[gi])
                        nc.scalar.activation(
                            out=gsb[:, :, :N], in_=ps[:, :, :N], func=FUNC[gi])
                        gate_sb[gi] = gsb
                    c_sb = gate_p.tile([TS, KT, 512], F32, name="csb", tag="g3")
                    nc.vector.tensor_mul(c_sb[:, :, :N], gate_sb[0][:, :, :N], gate_sb[3][:, :, :N])
                    if lvl < DEPTH:
                        nc.vector.tensor_add(c_sb[:, :, :N], c_sb[:, :, :N], cf_in[:, :, c0:c0 + N])
                    tc_sb = gate_p.tile([TS, KT, 512], F32, name="tcsb", tag="g0")
                    nc.scalar.activation(out=tc_sb[:, :, :N], in_=c_sb[:, :, :N], func=AF.Tanh)
                    if lvl > 0:
                        h_sb = gate_p.tile([TS, KT, 512], F32, name="hsb", tag="g2")
                        nc.vector.tensor_mul(h_sb[:, :, :N], gate_sb[2][:, :, :N], tc_sb[:, :, :N])
                        fc_sb = gate_p.tile([TS, KT, 512], F32, name="fcsb", tag="g1")
                        nc.vector.tensor_mul(fc_sb[:, :, :N], gate_sb[1][:, :, :N], c_sb[:, :, :N])
                        hv = h_sb[:, :, :N].rearrange("p k (nb two g) -> p k nb two g", two=2, g=G)
                        fv = fc_sb[:, :, :N].rearrange("p k (nb two g) -> p k nb two g", two=2, g=G)
                        po = c0 // 2
                        cho = ch_out[:, :, po:po + N // 2].rearrange("p k (nb g) -> p k nb g", g=G)
                        cfo = cf_out[:, :, po:po + N // 2].rearrange("p k (nb g) -> p k nb g", g=G)
                        nc.vector.tensor_add(cho, hv[:, :, :, 0, :], hv[:, :, :, 1, :])
                        nc.vector.tensor_add(cfo, fv[:, :, :, 0, :], fv[:, :, :, 1, :])
                    else:
                        nc.vector.tensor_mul(
                            h_root[:TS, :, g * G:(g + 1) * G],
                            gate_sb[2][:, :, :G], tc_sb[:, :, :G])
                qbase += nq_lvl
                ch_in, cf_in = ch_out, cf_out

        # classifier + log_softmax + sum(logp[gold]) over all 128 trees
        ps_cls = ps_g.tile([L, BSH], F32, name="pscls", tag="psg")
        for k in range(KT):
            ksz = TS + 1 if k == 0 else TS
            nc.tensor.matmul(out=ps_cls, lhsT=wout_sb[:ksz, k, :],
                             rhs=h_root[:ksz, k, :],
                             start=(k == 0), stop=(k == KT - 1))
        logitsT = gate_p.tile([L, BSH], F32, name="logitsT", tag="mA")
        nc.vector.tensor_copy(logitsT, ps_cls)
        ps_lg = ps_g.tile([BSH, L], F32, name="pslg", tag="psg")
        nc.tensor.transpose(out=ps_lg, in_=logitsT, identity=ident5)
        logits = gate_p.tile([BSH, L], F32, name="logits", tag="mB")
        nc.vector.tensor_copy(logits, ps_lg)
        msb = gate_p.tile([BSH, 1], F32, name="msb", tag="m1")
        nc.vector.reduce_max(out=msb, in_=logits, axis=mybir.AxisListType.X)
        negm = gate_p.tile([BSH, 1], F32, name="negm", tag="m2")
        nc.vector.tensor_scalar_mul(negm, msb, -1.0)
        esb = gate_p.tile([BSH, L], F32, name="esb", tag="mC")
        nc.scalar.activation(out=esb, in_=logits, func=AF.Exp, bias=negm[:, 0:1])
        ssb = gate_p.tile([BSH, 1], F32, name="ssb", tag="m3")
        nc.vector.reduce_sum(out=ssb, in_=esb, axis=mybir.AxisListType.X)
        lssb = gate_p.tile([BSH, 1], F32, name="lssb", tag="m4")
        nc.scalar.activation(out=lssb, in_=ssb, func=AF.Ln)
        tot = gate_p.tile([BSH, 1], F32, name="tot", tag="m5")
        nc.vector.tensor_add(tot, msb, lssb)
        logp_sb = gate_p.tile([BSH, L], F32, name="logpsb", tag="mD")
        nc.vector.tensor_scalar_sub(logp_sb, logits, tot[:, 0:1])
        nc.sync.dma_start(out=logp_d, in_=logp_sb)
        prod = gate_p.tile([BSH, L], F32, name="prod", tag="mE")
        nc.vector.tensor_mul(prod, logp_sb, oneh_sb)
        ptree = gate_p.tile([BSH, 1], F32, name="ptree", tag="m6")
        nc.vector.reduce_sum(out=ptree, in_=prod, axis=mybir.AxisListType.X)
        ps_nll = ps_g.tile([1, 1], F32, name="psnll", tag="psg")
        nc.tensor.matmul(out=ps_nll, lhsT=ptree, rhs=ones_sb, start=True, stop=True)
        nll_sb = gate_p.tile([1, 1], F32, name="nllsb", tag="m7")
        nc.vector.tensor_copy(nll_sb, ps_nll)
        nc.sync.dma_start(out=nll_d, in_=nll_sb)
    nc.finalize()
    return nc


def _pad_ktiles(w, cols, bias=None):
    """[300, cols] fp32 -> [KT, 128, cols] bf16; bias goes in row 100 of K-tile 0."""
    out = np.zeros((KT, 128, cols), dtype=ml_dtypes.bfloat16)
    for k in range(KT):
        out[k, :TS, :] = w[k * TS:(k + 1) * TS, :].astype(ml_dtypes.bfloat16)
    if bias is not None:
        out[0, TS, :] = bias.astype(ml_dtypes.bfloat16)
    return out


def prep_in_maps(inputs):
    wi = np.asarray(inputs["word_idx"]).astype(np.int32)
    gold = np.asarray(inputs["gold"]).astype(np.int64)
    emb = np.ascontiguousarray(np.asarray(inputs["embedding"], dtype=np.float32))
    W4 = np.concatenate(
        [np.asarray(inputs["W_" + n], dtype=np.float32) for n in ("ix", "fx", "ox", "ux")],
        axis=1)
    Wh = np.concatenate(
        [np.asarray(inputs["W_" + n], dtype=np.float32) for n in ("ih", "oh", "uh")],
        axis=1)
    b4 = np.concatenate([
        np.asarray(inputs["b_ix"]) + np.asarray(inputs["b_ih"]),
        np.asarray(inputs["b_fx"]) + np.asarray(inputs["b_fh"]),
        np.asarray(inputs["b_ox"]) + np.asarray(inputs["b_oh"]),
        np.asarray(inputs["b_ux"]) + np.asarray(inputs["b_uh"]),
    ]).astype(np.float32)
    wout = np.asarray(inputs["W_out"], dtype=np.float32)
    bout = np.asarray(inputs["b_out"], dtype=np.float32)
    quads = quad_list()
    qa = np.array([q[0] for q in quads])
    qb = np.array([q[1] for q in quads])
    eye = np.eye(L, dtype=np.float32)

    w4_p = _pad_ktiles(W4, 1200, bias=b4)
    wh_p = _pad_ktiles(Wh, 900)
    wout_p = _pad_ktiles(wout, L, bias=bout)

    in_maps = []
    for c in range(NCORES):
        t0 = c * BSH
        gidx = np.empty((128, NPASS * NQ), np.int32)
        for g in range(NPASS):
            blk = wi[t0 + g * G: t0 + (g + 1) * G]               # [64, 127]
            gidx[0:G, g * NQ:(g + 1) * NQ] = blk[:, qa]
            gidx[G:2 * G, g * NQ:(g + 1) * NQ] = blk[:, qb]
        in_maps.append(dict(
            gidx=np.ascontiguousarray(gidx), emb=emb, w4=w4_p, wh=wh_p,
            wout=wout_p,
            onehot=np.ascontiguousarray(eye[gold[t0:t0 + BSH]])))
    return in_maps


_PROG = None


def _get_prog():
    global _PROG
    if _PROG is None:
        _PROG = build_program()
    return _PROG


def _assemble(results):
    logp = np.concatenate([results[c]["logp"] for c in range(NCORES)], axis=0)
    tot = sum(float(results[c]["nll"][0, 0]) for c in range(NCORES))
    loss = np.float32(-tot / B)
    return np.ascontiguousarray(logp.astype(np.float32)), loss


def kernel(**inputs):
    nc = _get_prog()
    in_maps = prep_in_maps(inputs)
    res = run_bass_kernel_spmd(nc, in_maps, list(range(NCORES)))
    return _assemble(res.results)


def kernel_profiled(**inputs):
    """Same as kernel() but with NTFF tracing; returns (outputs, exec_time_ns)."""
    nc = _get_prog()
    in_maps = prep_in_maps(inputs)
    res = run_bass_kernel_spmd(nc, in_maps, list(range(NCORES)), trace=True)
    return _assemble(res.results), res.exec_time_ns


# revision 16
# speedup vs baseline: 1.4528x; 1.0079x over previous
# BatchChildSumTreeLSTM on 8 Trainium2 NeuronCores (Bass/Tile).
#
# Strategy: data-parallel over the 1024 trees (128 per core); weights and the
# 50000x300 embedding table are replicated per core. Inside each core:
#   - embedding rows are fetched with indirect (gather) DMA, 128 rows per call
#     in "quad" layout: partition p = (node_pair_idx p//64, tree p%64)
#   - gathered [row, dim] tiles are PE-transposed to [dim, row] so all gate
#     matmuls keep the contraction dim on partitions (3 K-tiles of 100)
#   - the three 100-dim M-tiles of each gate land in the free dimension of one
#     3-bank PSUM tile [100, 3, 512], so each gate takes a single activation op
#   - gate biases ride the x-side matmul for free: the first K-tile carries an
#     extra ones-row in the moving operand and the bias row in the weights
#   - levels are processed bottom-up; only pairwise child sums (sum h, sum f*c)
#     cross levels.  own_c/own_h are zero in the reference (nodes are
#     processed before their own state is written), so f = sigmoid(x@W_fx +
#     b_fx + b_fh) and c = i*u + sum_children(f_j*c_j); W_fh drops out.
#   - matmul operands are bf16 (fp32 accumulate in PSUM); end-to-end logp
#     error vs the fp32 reference is ~4e-4 absolute on a ~1.75 scale.
# Outputs: per-core logp [128,5] and the per-core sum of logp[gold]; the host
# concatenates logp shards and finishes loss = -sum/1024.

import numpy as np
from contextlib import ExitStack

import ml_dtypes

import concourse.bacc as bacc
import concourse.bass as bass
import concourse.mybir as mybir
import concourse.tile as tile
from concourse.bass import IndirectOffsetOnAxis
from concourse.bass_utils import run_bass_kernel_spmd
from concourse.masks import make_identity

DT = mybir.dt
F32 = DT.float32
BF16 = DT.bfloat16

V, DIN, H, L = 50000, 300, 300, 5
B, DEPTH, NPT = 1024, 6, 127
NCORES, BSH = 8, 128
G = 64                    # trees per pass
NPASS = BSH // G          # 2
NQ = 64                   # gather quads per pass (63 node pairs + root dup)
KT = 3                    # dim tiles over 300: 3 x 100
TS = 100
AF = mybir.ActivationFunctionType

# gate order in w4: i, f, o, u ; wh columns: i, o, u
H_COL = {0: 0, 2: 300, 3: 600}
FUNC = {0: AF.Sigmoid, 1: AF.Sigmoid, 2: AF.Sigmoid, 3: AF.Tanh}
# sbuf slot-tag reuse pairs: (i,tc) (f,fc) (o,h) (u,c)
GTAG = {0: "g0", 1: "g1", 2: "g2", 3: "g3"}


def quad_list():
    """Node pairs in bottom-up processing order; 63 pairs + (0,0) for roots."""
    quads = []
    for lvl in range(DEPTH, 0, -1):
        s, e = 2 ** lvl - 1, 2 ** (lvl + 1) - 1
        nodes = list(range(s, e))
        for i in range(0, len(nodes), 2):
            quads.append((nodes[i], nodes[i + 1]))
    quads.append((0, 0))
    return quads


def build_program():
    nc = bacc.Bacc(trn_type="TRN2", target_bir_lowering=False, debug=False)

    gidx_d = nc.dram_tensor("gidx", [128, NPASS * NQ], DT.int32, kind="ExternalInput").ap()
    emb_d = nc.dram_tensor("emb", [V, DIN], F32, kind="ExternalInput").ap()
    w4_d = nc.dram_tensor("w4", [KT, 128, 1200], BF16, kind="ExternalInput").ap()
    wh_d = nc.dram_tensor("wh", [KT, 128, 900], BF16, kind="ExternalInput").ap()
    wout_d = nc.dram_tensor("wout", [KT, 128, L], BF16, kind="ExternalInput").ap()
    oneh_d = nc.dram_tensor("onehot", [BSH, L], F32, kind="ExternalInput").ap()
    logp_d = nc.dram_tensor("logp", [BSH, L], F32, kind="ExternalOutput").ap()
    nll_d = nc.dram_tensor("nll", [1, 1], F32, kind="ExternalOutput").ap()

    with ExitStack() as ctx:
        tc = ctx.enter_context(tile.TileContext(nc))
        singles = ctx.enter_context(tc.tile_pool(name="singles", bufs=1))
        graw_p = ctx.enter_context(tc.tile_pool(name="grawp", bufs=10))
        xt_p = ctx.enter_context(tc.tile_pool(name="xtp", bufs=5))
        gate_p = ctx.enter_context(tc.tile_pool(name="gatep", bufs=3))
        state_p = ctx.enter_context(tc.tile_pool(name="statep", bufs=1))
        ps_g = ctx.enter_context(tc.tile_pool(name="psg", bufs=2, space="PSUM"))
        ps_t = ctx.enter_context(tc.tile_pool(name="pst", bufs=2, space="PSUM"))

        ident = singles.tile([128, 128], F32, name="ident")
        make_identity(nc, ident)
        ident5 = singles.tile([L, L], F32, name="ident5")
        make_identity(nc, ident5)
        w4_sb = singles.tile([128, KT, 1200], BF16, name="w4sb")
        nc.sync.dma_start(out=w4_sb, in_=w4_d.rearrange("k p c -> p k c"))
        wh_sb = singles.tile([128, KT, 900], BF16, name="whsb")
        nc.sync.dma_start(out=wh_sb, in_=wh_d.rearrange("k p c -> p k c"))
        wout_sb = singles.tile([128, KT, L], BF16, name="woutsb")
        nc.sync.dma_start(out=wout_sb, in_=wout_d.rearrange("k p c -> p k c"))
        oneh_sb = singles.tile([BSH, L], F32, name="onehsb")
        nc.sync.dma_start(out=oneh_sb, in_=oneh_d)
        gidx_sb = singles.tile([128, NPASS * NQ], DT.int32, name="gidxsb")
        nc.sync.dma_start(out=gidx_sb, in_=gidx_d)
        ones_sb = singles.tile([BSH, 1], F32, name="onessb")
        nc.vector.memset(ones_sb, 1.0)

        # h_root[100, 0, :] is a ones-row so the classifier bias rides the matmul
        h_root = state_p.tile([128, KT, BSH], BF16, name="hroot", tag="hroot")
        nc.vector.memset(h_root[96:128, 0, :], 1.0)  # rows 96:100 later overwritten by h

        for g in range(NPASS):
            qbase = 0
            ch_in = cf_in = None
            for lvl in range(DEPTH, -1, -1):
                nl = 2 ** lvl
                rows = nl * G if lvl > 0 else G
                nq_lvl = max(nl // 2, 1)
                if lvl > 0:
                    par = "A" if lvl % 2 == 0 else "B"
                    prow = (nl // 2) * G
                    ch_out = state_p.tile([TS, KT, prow], BF16, name=f"ch_{g}_{lvl}", tag="ch" + par)
                    cf_out = state_p.tile([TS, KT, prow], F32, name=f"cf_{g}_{lvl}", tag="cf" + par)
                else:
                    ch_out = cf_out = None
                nchunk = max(rows // 512, 1)
                qpc = nq_lvl // nchunk
                N = min(rows, 512)
                for c in range(nchunk):
                    q0 = g * NQ + qbase + c * qpc
                    graw = graw_p.tile([128, 4, DIN], F32, name="graw", tag="graw")
                    for qi in range(qpc):
                        # one gather per quad: [128,1] index AP is the only
                        # form the HW descriptor unroll handles correctly
                        nc.gpsimd.indirect_dma_start(
                            out=graw[:, qi, :], out_offset=None,
                            in_=emb_d,
                            in_offset=IndirectOffsetOnAxis(
                                ap=gidx_sb[:, q0 + qi:q0 + qi + 1], axis=0),
                        )
                    xt = xt_p.tile([128, KT, 512], BF16, name="xt", tag="xt")
                    nc.vector.memset(xt[96:128, 0, :], 1.0)  # ones-row at 100; 96:100 overwritten by cast
                    ncols = 128 if lvl > 0 else 64
                    for qi in range(qpc):
                        pst = ps_t.tile([TS, KT * 128], F32, name="pst", tag="pst")
                        for k in range(KT):
                            nc.tensor.transpose(
                                out=pst[:, k * 128:(k + 1) * 128],
                                in_=graw[:, qi, k * TS:(k + 1) * TS],
                                identity=ident)
                        # single fp32->bf16 cast per quad (3 K-tiles at once)
                        nc.vector.tensor_copy(
                            out=xt[:TS, :, qi * 128: qi * 128 + ncols],
                            in_=pst.rearrange("p (k c) -> p k c", k=KT)[:, :, :ncols])
                    c0 = c * 512
                    gate_sb = {}
                    for gi in range(4):
                        if lvl == 0 and gi == 1:
                            continue
                        ps = ps_g.tile([TS, KT, 512], F32, name="psgate", tag="psg")
                        has_h = (lvl < DEPTH) and (gi in H_COL)
                        for m in range(KT):
                            for k in range(KT):
                                ksz = TS + 1 if k == 0 else TS  # ones/bias row
                                nc.tensor.matmul(
                                    out=ps[:, m, :N],
                                    lhsT=w4_sb[:ksz, k, gi * 300 + m * TS: gi * 300 + (m + 1) * TS],
                                    rhs=xt[:ksz, k, :N],
                                    start=(k == 0), stop=(k == KT - 1 and not has_h))
                            if has_h:
                                for k in range(KT):
                                    nc.tensor.matmul(
                                        out=ps[:, m, :N],
                                        lhsT=wh_sb[:TS, k, H_COL[gi] + m * TS: H_COL[gi] + (m + 1) * TS],
                                        rhs=ch_in[:, k, c0:c0 + N],
                                        start=False, stop=(k == KT - 1))
                        gsb = gate_p.tile([TS, KT, 512], F32, name=f"g{gi}", tag=GTAG[gi])
                        nc.scalar.activation(
                            out=gsb[:, :, :N], in_=ps[:, :, :N], func=FUNC[gi])
                        gate_sb[gi] = gsb
                    c_sb = gate_p.tile([TS, KT, 512], F32, name="csb", tag="g3")
                    nc.vector.tensor_mul(c_sb[:, :, :N], gate_sb[0][:, :, :N], gate_sb[3][:, :, :N])
                    if lvl < DEPTH:
                        nc.vector.tensor_add(c_sb[:, :, :N], c_sb[:, :, :N], cf_in[:, :, c0:c0 + N])
                    tc_sb = gate_p.tile([TS, KT, 512], F32, name="tcsb", tag="g0")
                    nc.scalar.activation(out=tc_sb[:, :, :N], in_=c_sb[:, :, :N], func=AF.Tanh)
                    if lvl > 0:
                        h_sb = gate_p.tile([TS, KT, 512], F32, name="hsb", tag="g2")
                        nc.vector.tensor_mul(h_sb[:, :, :N], gate_sb[2][:, :, :N], tc_sb[:, :, :N])
                        fc_sb = gate_p.tile([TS, KT, 512], F32, name="fcsb", tag="g1")
                        nc.vector.tensor_mul(fc_sb[:, :, :N], gate_sb[1][:, :, :N], c_sb[:, :, :N])
                        hv = h_sb[:, :, :N].rearrange("p k (nb two g) -> p k nb two g", two=2, g=G)
                        fv = fc_sb[:, :, :N].rearrange("p k (nb two g) -> p k nb two g", two=2, g=G)
                        po = c0 // 2
                        cho = ch_out[:, :, po:po + N // 2].rearrange("p k (nb g) -> p k nb g", g=G)
                        cfo = cf_out[:, :, po:po + N // 2].rearrange("p k (nb g) -> p k nb g", g=G)
                        nc.vector.tensor_add(cho, hv[:, :, :, 0, :], hv[:, :, :, 1, :])
                        nc.vector.tensor_add(cfo, fv[:, :, :, 0, :], fv[:, :, :, 1, :])
                    else:
                        nc.vector.tensor_mul(
                            h_root[:TS, :, g * G:(g + 1) * G],
                            gate_sb[2][:, :, :G], tc_sb[:, :, :G])
                qbase += nq_lvl
                ch_in, cf_in = ch_out, cf_out

        # classifier + log_softmax + sum(logp[gold]) over all 128 trees
        ps_cls = ps_g.tile([L, BSH], F32, name="pscls", tag="psg")
        for k in range(KT):
            ksz = TS + 1 if k == 0 else TS
            nc.tensor.matmul(out=ps_cls, lhsT=wout_sb[:ksz, k, :],
                             rhs=h_root[:ksz, k, :],
                             start=(k == 0), stop=(k == KT - 1))
        logitsT = gate_p.tile([L, BSH], F32, name="logitsT", tag="mA")
        nc.vector.tensor_copy(logitsT, ps_cls)
        ps_lg = ps_g.tile([BSH, L], F32, name="pslg", tag="psg")
        nc.tensor.transpose(out=ps_lg, in_=logitsT, identity=ident5)
        logits = gate_p.tile([BSH, L], F32, name="logits", tag="mB")
        nc.vector.tensor_copy(logits, ps_lg)
        msb = gate_p.tile([BSH, 1], F32, name="msb", tag="m1")
        nc.vector.reduce_max(out=msb, in_=logits, axis=mybir.AxisListType.X)
        negm = gate_p.tile([BSH, 1], F32, name="negm", tag="m2")
        nc.vector.tensor_scalar_mul(negm, msb, -1.0)
        esb = gate_p.tile([BSH, L], F32, name="esb", tag="mC")
        nc.scalar.activation(out=esb, in_=logits, func=AF.Exp, bias=negm[:, 0:1])
        ssb = gate_p.tile([BSH, 1], F32, name="ssb", tag="m3")
        nc.vector.reduce_sum(out=ssb, in_=esb, axis=mybir.AxisListType.X)
        lssb = gate_p.tile([BSH, 1], F32, name="lssb", tag="m4")
        nc.scalar.activation(out=lssb, in_=ssb, func=AF.Ln)
        tot = gate_p.tile([BSH, 1], F32, name="tot", tag="m5")
        nc.vector.tensor_add(tot, msb, lssb)
        logp_sb = gate_p.tile([BSH, L], F32, name="logpsb", tag="mD")
        nc.vector.tensor_scalar_sub(logp_sb, logits, tot[:, 0:1])
        nc.sync.dma_start(out=logp_d, in_=logp_sb)
        prod = gate_p.tile([BSH, L], F32, name="prod", tag="mE")
        nc.vector.tensor_mul(prod, logp_sb, oneh_sb)
        ptree = gate_p.tile([BSH, 1], F32, name="ptree", tag="m6")
        nc.vector.reduce_sum(out=ptree, in_=prod, axis=mybir.AxisListType.X)
        ps_nll = ps_g.tile([1, 1], F32, name="psnll", tag="psg")
        nc.tensor.matmul(out=ps_nll, lhsT=ptree, rhs=ones_sb, start=True, stop=True)
        nll_sb = gate_p.tile([1, 1], F32, name="nllsb", tag="m7")
        nc.vector.tensor_copy(nll_sb, ps_nll)
        nc.sync.dma_start(out=nll_d, in_=nll_sb)
    nc.finalize()
    return nc


def _pad_ktiles(w, cols, bias=None):
    """[300, cols] fp32 -> [KT, 128, cols] bf16; bias goes in row 100 of K-tile 0."""
    out = np.zeros((KT, 128, cols), dtype=ml_dtypes.bfloat16)
    for k in range(KT):
        out[k, :TS, :] = w[k * TS:(k + 1) * TS, :].astype(ml_dtypes.bfloat16)
    if bias is not None:
        out[0, TS, :] = bias.astype(ml_dtypes.bfloat16)
    return out


def prep_in_maps(inputs):
    wi = np.asarray(inputs["word_idx"]).astype(np.int32)
    gold = np.asarray(inputs["gold"]).astype(np.int64)
    emb = np.ascontiguousarray(np.asarray(inputs["embedding"], dtype=np.float32))
    W4 = np.concatenate(
        [np.asarray(inputs["W_" + n], dtype=np.float32) for n in ("ix", "fx", "ox", "ux")],
        axis=1)
    Wh = np.concatenate(
        [np.asarray(inputs["W_" + n], dtype=np.float32) for n in ("ih", "oh", "uh")],
        axis=1)
    b4 = np.concatenate([
        np.asarray(inputs["b_ix"]) + np.asarray(inputs["b_ih"]),
        np.asarray(inputs["b_fx"]) + np.asarray(inputs["b_fh"]),
        np.asarray(inputs["b_ox"]) + np.asarray(inputs["b_oh"]),
        np.asarray(inputs["b_ux"]) + np.asarray(inputs["b_uh"]),
    ]).astype(np.float32)
    wout = np.asarray(inputs["W_out"], dtype=np.float32)
    bout = np.asarray(inputs["b_out"], dtype=np.float32)
    quads = quad_list()
    qa = np.array([q[0] for q in quads])
    qb = np.array([q[1] for q in quads])
    eye = np.eye(L, dtype=np.float32)

    w4_p = _pad_ktiles(W4, 1200, bias=b4)
    wh_p = _pad_ktiles(Wh, 900)
    wout_p = _pad_ktiles(wout, L, bias=bout)

    in_maps = []
    for c in range(NCORES):
        t0 = c * BSH
        gidx = np.empty((128, NPASS * NQ), np.int32)
        for g in range(NPASS):
            blk = wi[t0 + g * G: t0 + (g + 1) * G]               # [64, 127]
            gidx[0:G, g * NQ:(g + 1) * NQ] = blk[:, qa]
            gidx[G:2 * G, g * NQ:(g + 1) * NQ] = blk[:, qb]
        in_maps.append(dict(
            gidx=np.ascontiguousarray(gidx), emb=emb, w4=w4_p, wh=wh_p,
            wout=wout_p,
            onehot=np.ascontiguousarray(eye[gold[t0:t0 + BSH]])))
    return in_maps


_PROG = None


def _get_prog():
    global _PROG
    if _PROG is None:
        _PROG = build_program()
    return _PROG


def _assemble(results):
    logp = np.concatenate([results[c]["logp"] for c in range(NCORES)], axis=0)
    tot = sum(float(results[c]["nll"][0, 0]) for c in range(NCORES))
    loss = np.float32(-tot / B)
    return np.ascontiguousarray(logp.astype(np.float32)), loss


def kernel(**inputs):
    nc = _get_prog()
    in_maps = prep_in_maps(inputs)
    res = run_bass_kernel_spmd(nc, in_maps, list(range(NCORES)))
    return _assemble(res.results)


def kernel_profiled(**inputs):
    """Same as kernel() but with NTFF tracing; returns (outputs, exec_time_ns)."""
    nc = _get_prog()
    in_maps = prep_in_maps(inputs)
    res = run_bass_kernel_spmd(nc, in_maps, list(range(NCORES)), trace=True)
    return _assemble(res.results), res.exec_time_ns


# revision 17
# speedup vs baseline: 1.5570x; 1.0717x over previous
# BatchChildSumTreeLSTM on 8 Trainium2 NeuronCores (Bass/Tile).
#
# Strategy: data-parallel over the 1024 trees (128 per core); weights and the
# 50000x300 embedding table are replicated per core. Inside each core:
#   - embedding rows are fetched with indirect (gather) DMA, 128 rows per call
#     in "quad" layout: partition p = (node_pair_idx p//64, tree p%64)
#   - gathered [row, dim] tiles are PE-transposed to [dim, row] so all gate
#     matmuls keep the contraction dim on partitions (3 K-tiles of 100)
#   - the three 100-dim M-tiles of each gate land in the free dimension of one
#     3-bank PSUM tile [100, 3, 512], so each gate takes a single activation op
#   - gate biases ride the x-side matmul for free: the first K-tile carries an
#     extra ones-row in the moving operand and the bias row in the weights
#   - levels are processed bottom-up; only pairwise child sums (sum h, sum f*c)
#     cross levels.  own_c/own_h are zero in the reference (nodes are
#     processed before their own state is written), so f = sigmoid(x@W_fx +
#     b_fx + b_fh) and c = i*u + sum_children(f_j*c_j); W_fh drops out.
#   - matmul operands are bf16 (fp32 accumulate in PSUM); end-to-end logp
#     error vs the fp32 reference is ~4e-4 absolute on a ~1.75 scale.
# Outputs: per-core logp [128,5] and the per-core sum of logp[gold]; the host
# concatenates logp shards and finishes loss = -sum/1024.

import numpy as np
from contextlib import ExitStack

import ml_dtypes

import concourse.bacc as bacc
import concourse.bass as bass
import concourse.mybir as mybir
import concourse.tile as tile
from concourse.bass import IndirectOffsetOnAxis
from concourse.bass_utils import run_bass_kernel_spmd
from concourse.masks import make_identity

DT = mybir.dt
F32 = DT.float32
BF16 = DT.bfloat16

V, DIN, H, L = 50000, 300, 300, 5
B, DEPTH, NPT = 1024, 6, 127
NCORES, BSH = 8, 128
G = 128                   # trees per pass (single pass)
NPASS = BSH // G          # 1
NQ = 127                  # gather calls per pass: one per node
KT = 3                    # dim tiles over 300: 3 x 100
TS = 100
AF = mybir.ActivationFunctionType

# gate order in w4: i, f, o, u ; wh columns: i, o, u
H_COL = {0: 0, 2: 300, 3: 600}
FUNC = {0: AF.Sigmoid, 1: AF.Sigmoid, 2: AF.Sigmoid, 3: AF.Tanh}
# sbuf slot-tag reuse pairs: (i,tc) (f,fc) (o,h) (u,c)
GTAG = {0: "g0", 1: "g1", 2: "g2", 3: "g3"}


def node_list():
    """All 127 nodes in bottom-up level order (leaves first, root last)."""
    nodes = []
    for lvl in range(DEPTH, -1, -1):
        nodes.extend(range(2 ** lvl - 1, 2 ** (lvl + 1) - 1))
    return nodes


def build_program():
    nc = bacc.Bacc(trn_type="TRN2", target_bir_lowering=False, debug=False)

    gidx_d = nc.dram_tensor("gidx", [128, NPASS * NQ], DT.int32, kind="ExternalInput").ap()
    emb_d = nc.dram_tensor("emb", [V, DIN], F32, kind="ExternalInput").ap()
    w4_d = nc.dram_tensor("w4", [KT, 128, 1200], BF16, kind="ExternalInput").ap()
    wh_d = nc.dram_tensor("wh", [KT, 128, 900], BF16, kind="ExternalInput").ap()
    wout_d = nc.dram_tensor("wout", [KT, 128, L], BF16, kind="ExternalInput").ap()
    oneh_d = nc.dram_tensor("onehot", [BSH, L], F32, kind="ExternalInput").ap()
    logp_d = nc.dram_tensor("logp", [BSH, L], F32, kind="ExternalOutput").ap()
    nll_d = nc.dram_tensor("nll", [1, 1], F32, kind="ExternalOutput").ap()

    with ExitStack() as ctx:
        tc = ctx.enter_context(tile.TileContext(nc))
        singles = ctx.enter_context(tc.tile_pool(name="singles", bufs=1))
        graw_p = ctx.enter_context(tc.tile_pool(name="grawp", bufs=8))
        xt_p = ctx.enter_context(tc.tile_pool(name="xtp", bufs=4))
        gate_p = ctx.enter_context(tc.tile_pool(name="gatep", bufs=2))
        state_p = ctx.enter_context(tc.tile_pool(name="statep", bufs=1))
        ps_g = ctx.enter_context(tc.tile_pool(name="psg", bufs=2, space="PSUM"))
        ps_t = ctx.enter_context(tc.tile_pool(name="pst", bufs=2, space="PSUM"))

        ident = singles.tile([128, 128], F32, name="ident")
        make_identity(nc, ident)
        ident5 = singles.tile([L, L], F32, name="ident5")
        make_identity(nc, ident5)
        w4_sb = singles.tile([128, KT, 1200], BF16, name="w4sb")
        nc.sync.dma_start(out=w4_sb, in_=w4_d.rearrange("k p c -> p k c"))
        wh_sb = singles.tile([128, KT, 900], BF16, name="whsb")
        nc.sync.dma_start(out=wh_sb, in_=wh_d.rearrange("k p c -> p k c"))
        wout_sb = singles.tile([128, KT, L], BF16, name="woutsb")
        nc.sync.dma_start(out=wout_sb, in_=wout_d.rearrange("k p c -> p k c"))
        oneh_sb = singles.tile([BSH, L], F32, name="onehsb")
        nc.sync.dma_start(out=oneh_sb, in_=oneh_d)
        gidx_sb = singles.tile([128, NPASS * NQ], DT.int32, name="gidxsb")
        nc.sync.dma_start(out=gidx_sb, in_=gidx_d)
        ones_sb = singles.tile([BSH, 1], F32, name="onessb")
        nc.vector.memset(ones_sb, 1.0)

        # h_root[100, 0, :] is a ones-row so the classifier bias rides the matmul
        h_root = state_p.tile([128, KT, BSH], BF16, name="hroot", tag="hroot")
        nc.vector.memset(h_root[96:128, 0, :], 1.0)  # rows 96:100 later overwritten by h

        for g in range(NPASS):
            qbase = 0
            ch_in = cf_in = None
            for lvl in range(DEPTH, -1, -1):
                nl = 2 ** lvl
                rows = nl * G
                nq_lvl = nl
                if lvl > 0:
                    par = "A" if lvl % 2 == 0 else "B"
                    prow = (nl // 2) * G
                    ch_out = state_p.tile([TS, KT, prow], BF16, name=f"ch_{g}_{lvl}", tag="ch" + par)
                    cf_out = state_p.tile([TS, KT, prow], BF16, name=f"cf_{g}_{lvl}", tag="cf" + par)
                else:
                    ch_out = cf_out = None
                nchunk = max(rows // 512, 1)
                qpc = nq_lvl // nchunk
                N = min(rows, 512)
                for c in range(nchunk):
                    q0 = g * NQ + qbase + c * qpc
                    graw = graw_p.tile([128, 4, DIN], F32, name="graw", tag="graw")
                    for qi in range(qpc):
                        # one gather per quad: [128,1] index AP is the only
                        # form the HW descriptor unroll handles correctly
                        nc.gpsimd.indirect_dma_start(
                            out=graw[:, qi, :], out_offset=None,
                            in_=emb_d,
                            in_offset=IndirectOffsetOnAxis(
                                ap=gidx_sb[:, q0 + qi:q0 + qi + 1], axis=0),
                        )
                    xt = xt_p.tile([128, KT, 512], BF16, name="xt", tag="xt")
                    nc.vector.memset(xt[96:128, 0, :], 1.0)  # ones-row at 100; 96:100 overwritten by cast
                    for qi in range(qpc):
                        pst = ps_t.tile([TS, KT * 128], F32, name="pst", tag="pst")
                        for k in range(KT):
                            nc.tensor.transpose(
                                out=pst[:, k * 128:(k + 1) * 128],
                                in_=graw[:, qi, k * TS:(k + 1) * TS],
                                identity=ident)
                        # single fp32->bf16 cast per quad (3 K-tiles at once)
                        nc.vector.tensor_copy(
                            out=xt[:TS, :, qi * 128: (qi + 1) * 128],
                            in_=pst.rearrange("p (k c) -> p k c", k=KT)[:, :, :])
                    c0 = c * 512
                    gate_sb = {}
                    for gi in range(4):
                        if lvl == 0 and gi == 1:
                            continue
                        ps = ps_g.tile([TS, KT, 512], F32, name="psgate", tag="psg")
                        has_h = (lvl < DEPTH) and (gi in H_COL)
                        for m in range(KT):
                            for k in range(KT):
                                ksz = TS + 1 if k == 0 else TS  # ones/bias row
                                nc.tensor.matmul(
                                    out=ps[:, m, :N],
                                    lhsT=w4_sb[:ksz, k, gi * 300 + m * TS: gi * 300 + (m + 1) * TS],
                                    rhs=xt[:ksz, k, :N],
                                    start=(k == 0), stop=(k == KT - 1 and not has_h))
                            if has_h:
                                for k in range(KT):
                                    nc.tensor.matmul(
                                        out=ps[:, m, :N],
                                        lhsT=wh_sb[:TS, k, H_COL[gi] + m * TS: H_COL[gi] + (m + 1) * TS],
                                        rhs=ch_in[:, k, c0:c0 + N],
                                        start=False, stop=(k == KT - 1))
                        gsb = gate_p.tile([TS, KT, 512], F32, name=f"g{gi}", tag=GTAG[gi])
                        nc.scalar.activation(
                            out=gsb[:, :, :N], in_=ps[:, :, :N], func=FUNC[gi])
                        gate_sb[gi] = gsb
                    c_sb = gate_p.tile([TS, KT, 512], F32, name="csb", tag="g3")
                    nc.vector.tensor_mul(c_sb[:, :, :N], gate_sb[0][:, :, :N], gate_sb[3][:, :, :N])
                    if lvl < DEPTH:
                        nc.vector.tensor_add(c_sb[:, :, :N], c_sb[:, :, :N], cf_in[:, :, c0:c0 + N])
                    tc_sb = gate_p.tile([TS, KT, 512], F32, name="tcsb", tag="g0")
                    nc.scalar.activation(out=tc_sb[:, :, :N], in_=c_sb[:, :, :N], func=AF.Tanh)
                    if lvl > 0:
                        h_sb = gate_p.tile([TS, KT, 512], F32, name="hsb", tag="g2")
                        nc.vector.tensor_mul(h_sb[:, :, :N], gate_sb[2][:, :, :N], tc_sb[:, :, :N])
                        fc_sb = gate_p.tile([TS, KT, 512], F32, name="fcsb", tag="g1")
                        nc.vector.tensor_mul(fc_sb[:, :, :N], gate_sb[1][:, :, :N], c_sb[:, :, :N])
                        hv = h_sb[:, :, :N].rearrange("p k (nb two g) -> p k nb two g", two=2, g=G)
                        fv = fc_sb[:, :, :N].rearrange("p k (nb two g) -> p k nb two g", two=2, g=G)
                        po = c0 // 2
                        cho = ch_out[:, :, po:po + N // 2].rearrange("p k (nb g) -> p k nb g", g=G)
                        cfo = cf_out[:, :, po:po + N // 2].rearrange("p k (nb g) -> p k nb g", g=G)
                        nc.vector.tensor_add(cho, hv[:, :, :, 0, :], hv[:, :, :, 1, :])
                        nc.vector.tensor_add(cfo, fv[:, :, :, 0, :], fv[:, :, :, 1, :])
                    else:
                        nc.vector.tensor_mul(
                            h_root[:TS, :, g * G:(g + 1) * G],
                            gate_sb[2][:, :, :G], tc_sb[:, :, :G])
                qbase += nq_lvl
                ch_in, cf_in = ch_out, cf_out

        # classifier + log_softmax + sum(logp[gold]) over all 128 trees
        ps_cls = ps_g.tile([L, BSH], F32, name="pscls", tag="psg")
        for k in range(KT):
            ksz = TS + 1 if k == 0 else TS
            nc.tensor.matmul(out=ps_cls, lhsT=wout_sb[:ksz, k, :],
                             rhs=h_root[:ksz, k, :],
                             start=(k == 0), stop=(k == KT - 1))
        logitsT = gate_p.tile([L, BSH], F32, name="logitsT", tag="mA")
        nc.vector.tensor_copy(logitsT, ps_cls)
        ps_lg = ps_g.tile([BSH, L], F32, name="pslg", tag="psg")
        nc.tensor.transpose(out=ps_lg, in_=logitsT, identity=ident5)
        logits = gate_p.tile([BSH, L], F32, name="logits", tag="mB")
        nc.vector.tensor_copy(logits, ps_lg)
        msb = gate_p.tile([BSH, 1], F32, name="msb", tag="m1")
        nc.vector.reduce_max(out=msb, in_=logits, axis=mybir.AxisListType.X)
        negm = gate_p.tile([BSH, 1], F32, name="negm", tag="m2")
        nc.vector.tensor_scalar_mul(negm, msb, -1.0)
        esb = gate_p.tile([BSH, L], F32, name="esb", tag="mC")
        nc.scalar.activation(out=esb, in_=logits, func=AF.Exp, bias=negm[:, 0:1])
        ssb = gate_p.tile([BSH, 1], F32, name="ssb", tag="m3")
        nc.vector.reduce_sum(out=ssb, in_=esb, axis=mybir.AxisListType.X)
        lssb = gate_p.tile([BSH, 1], F32, name="lssb", tag="m4")
        nc.scalar.activation(out=lssb, in_=ssb, func=AF.Ln)
        tot = gate_p.tile([BSH, 1], F32, name="tot", tag="m5")
        nc.vector.tensor_add(tot, msb, lssb)
        logp_sb = gate_p.tile([BSH, L], F32, name="logpsb", tag="mD")
        nc.vector.tensor_scalar_sub(logp_sb, logits, tot[:, 0:1])
        nc.sync.dma_start(out=logp_d, in_=logp_sb)
        prod = gate_p.tile([BSH, L], F32, name="prod", tag="mE")
        nc.vector.tensor_mul(prod, logp_sb, oneh_sb)
        ptree = gate_p.tile([BSH, 1], F32, name="ptree", tag="m6")
        nc.vector.reduce_sum(out=ptree, in_=prod, axis=mybir.AxisListType.X)
        ps_nll = ps_g.tile([1, 1], F32, name="psnll", tag="psg")
        nc.tensor.matmul(out=ps_nll, lhsT=ptree, rhs=ones_sb, start=True, stop=True)
        nll_sb = gate_p.tile([1, 1], F32, name="nllsb", tag="m7")
        nc.vector.tensor_copy(nll_sb, ps_nll)
        nc.sync.dma_start(out=nll_d, in_=nll_sb)
    nc.finalize()
    return nc


def _pad_ktiles(w, cols, bias=None):
    """[300, cols] fp32 -> [KT, 128, cols] bf16; bias goes in row 100 of K-tile 0."""
    out = np.zeros((KT, 128, cols), dtype=ml_dtypes.bfloat16)
    for k in range(KT):
        out[k, :TS, :] = w[k * TS:(k + 1) * TS, :].astype(ml_dtypes.bfloat16)
    if bias is not None:
        out[0, TS, :] = bias.astype(ml_dtypes.bfloat16)
    return out


def prep_in_maps(inputs):
    wi = np.asarray(inputs["word_idx"]).astype(np.int32)
    gold = np.asarray(inputs["gold"]).astype(np.int64)
    emb = np.ascontiguousarray(np.asarray(inputs["embedding"], dtype=np.float32))
    W4 = np.concatenate(
        [np.asarray(inputs["W_" + n], dtype=np.float32) for n in ("ix", "fx", "ox", "ux")],
        axis=1)
    Wh = np.concatenate(
        [np.asarray(inputs["W_" + n], dtype=np.float32) for n in ("ih", "oh", "uh")],
        axis=1)
    b4 = np.concatenate([
        np.asarray(inputs["b_ix"]) + np.asarray(inputs["b_ih"]),
        np.asarray(inputs["b_fx"]) + np.asarray(inputs["b_fh"]),
        np.asarray(inputs["b_ox"]) + np.asarray(inputs["b_oh"]),
        np.asarray(inputs["b_ux"]) + np.asarray(inputs["b_uh"]),
    ]).astype(np.float32)
    wout = np.asarray(inputs["W_out"], dtype=np.float32)
    bout = np.asarray(inputs["b_out"], dtype=np.float32)
    norder = np.array(node_list())
    eye = np.eye(L, dtype=np.float32)

    w4_p = _pad_ktiles(W4, 1200, bias=b4)
    wh_p = _pad_ktiles(Wh, 900)
    wout_p = _pad_ktiles(wout, L, bias=bout)

    in_maps = []
    for c in range(NCORES):
        t0 = c * BSH
        gidx = np.ascontiguousarray(wi[t0:t0 + BSH][:, norder])  # [128, 127]
        in_maps.append(dict(
            gidx=gidx, emb=emb, w4=w4_p, wh=wh_p,
            wout=wout_p,
            onehot=np.ascontiguousarray(eye[gold[t0:t0 + BSH]])))
    return in_maps


_PROG = None


def _get_prog():
    global _PROG
    if _PROG is None:
        _PROG = build_program()
    return _PROG


def _assemble(results):
    logp = np.concatenate([results[c]["logp"] for c in range(NCORES)], axis=0)
    tot = sum(float(results[c]["nll"][0, 0]) for c in range(NCORES))
    loss = np.float32(-tot / B)
    return np.ascontiguousarray(logp.astype(np.float32)), loss


def kernel(**inputs):
    nc = _get_prog()
    in_maps = prep_in_maps(inputs)
    res = run_bass_kernel_spmd(nc, in_maps, list(range(NCORES)))
    return _assemble(res.results)


def kernel_profiled(**inputs):
    """Same as kernel() but with NTFF tracing; returns (outputs, exec_time_ns)."""
    nc = _get_prog()
    in_maps = prep_in_maps(inputs)
    res = run_bass_kernel_spmd(nc, in_maps, list(range(NCORES)), trace=True)
    return _assemble(res.results), res.exec_time_ns


# revision 22
# speedup vs baseline: 1.5688x; 1.0075x over previous
# BatchChildSumTreeLSTM on 8 Trainium2 NeuronCores (Bass/Tile).
#
# Strategy: data-parallel over the 1024 trees (128 per core); weights and the
# 50000x300 embedding table are replicated per core. Inside each core:
#   - embedding rows are fetched with indirect (gather) DMA, 128 rows per call
#     in "quad" layout: partition p = (node_pair_idx p//64, tree p%64)
#   - gathered [row, dim] tiles are PE-transposed to [dim, row] so all gate
#     matmuls keep the contraction dim on partitions (3 K-tiles of 100)
#   - the three 100-dim M-tiles of each gate land in the free dimension of one
#     3-bank PSUM tile [100, 3, 512], so each gate takes a single activation op
#   - gate biases ride the x-side matmul for free: the first K-tile carries an
#     extra ones-row in the moving operand and the bias row in the weights
#   - levels are processed bottom-up; only pairwise child sums (sum h, sum f*c)
#     cross levels.  own_c/own_h are zero in the reference (nodes are
#     processed before their own state is written), so f = sigmoid(x@W_fx +
#     b_fx + b_fh) and c = i*u + sum_children(f_j*c_j); W_fh drops out.
#   - matmul operands are bf16 (fp32 accumulate in PSUM); end-to-end logp
#     error vs the fp32 reference is ~4e-4 absolute on a ~1.75 scale.
# Outputs: per-core logp [128,5] and the per-core sum of logp[gold]; the host
# concatenates logp shards and finishes loss = -sum/1024.

import numpy as np
from contextlib import ExitStack

import ml_dtypes

import concourse.bacc as bacc
import concourse.bass as bass
import concourse.mybir as mybir
import concourse.tile as tile
from concourse.bass import IndirectOffsetOnAxis
from concourse.bass_utils import run_bass_kernel_spmd
from concourse.masks import make_identity

DT = mybir.dt
F32 = DT.float32
BF16 = DT.bfloat16

V, DIN, H, L = 50000, 300, 300, 5
B, DEPTH, NPT = 1024, 6, 127
NCORES, BSH = 8, 128
G = 128                   # trees per pass (single pass)
NPASS = BSH // G          # 1
NQ = 127                  # gather calls per pass: one per node
KT = 3                    # dim tiles over 300: 3 x 100
TS = 100
AF = mybir.ActivationFunctionType

# gate order in w4: i, f, o, u ; wh columns: i, o, u
H_COL = {0: 0, 2: 300, 3: 600}
FUNC = {0: AF.Sigmoid, 1: AF.Sigmoid, 2: AF.Sigmoid, 3: AF.Tanh}
# sbuf slot-tag reuse pairs: (i,tc) (f,fc) (o,h) (u,c)
GTAG = {0: "g0", 1: "g1", 2: "g2", 3: "g3"}


def node_list():
    """All 127 nodes in bottom-up level order (leaves first, root last)."""
    nodes = []
    for lvl in range(DEPTH, -1, -1):
        nodes.extend(range(2 ** lvl - 1, 2 ** (lvl + 1) - 1))
    return nodes


def build_program():
    nc = bacc.Bacc(trn_type="TRN2", target_bir_lowering=False, debug=False)

    gidx_d = nc.dram_tensor("gidx", [128, NPASS * NQ], DT.int32, kind="ExternalInput").ap()
    emb_d = nc.dram_tensor("emb", [V, DIN], F32, kind="ExternalInput").ap()
    w4_d = nc.dram_tensor("w4", [KT, 128, 1200], BF16, kind="ExternalInput").ap()
    wh_d = nc.dram_tensor("wh", [KT, 128, 900], BF16, kind="ExternalInput").ap()
    wout_d = nc.dram_tensor("wout", [KT, 128, L], BF16, kind="ExternalInput").ap()
    oneh_d = nc.dram_tensor("onehot", [BSH, L], F32, kind="ExternalInput").ap()
    logp_d = nc.dram_tensor("logp", [BSH, L], F32, kind="ExternalOutput").ap()
    nll_d = nc.dram_tensor("nll", [1, 1], F32, kind="ExternalOutput").ap()

    with ExitStack() as ctx:
        tc = ctx.enter_context(tile.TileContext(nc))
        singles = ctx.enter_context(tc.tile_pool(name="singles", bufs=1))
        graw_p = ctx.enter_context(tc.tile_pool(name="grawp", bufs=8))
        xt_p = ctx.enter_context(tc.tile_pool(name="xtp", bufs=4))
        gate_p = ctx.enter_context(tc.tile_pool(name="gatep", bufs=2))
        state_p = ctx.enter_context(tc.tile_pool(name="statep", bufs=1))
        ps_g = ctx.enter_context(tc.tile_pool(name="psg", bufs=2, space="PSUM"))
        ps_t = ctx.enter_context(tc.tile_pool(name="pst", bufs=2, space="PSUM"))

        ident = singles.tile([128, 128], F32, name="ident")
        make_identity(nc, ident)
        ident5 = singles.tile([L, L], F32, name="ident5")
        make_identity(nc, ident5)
        w4_sb = singles.tile([128, KT, 1200], BF16, name="w4sb")
        nc.sync.dma_start(out=w4_sb, in_=w4_d.rearrange("k p c -> p k c"))
        wh_sb = singles.tile([128, KT, 900], BF16, name="whsb")
        nc.sync.dma_start(out=wh_sb, in_=wh_d.rearrange("k p c -> p k c"))
        wout_sb = singles.tile([128, KT, L], BF16, name="woutsb")
        nc.sync.dma_start(out=wout_sb, in_=wout_d.rearrange("k p c -> p k c"))
        oneh_sb = singles.tile([BSH, L], F32, name="onehsb")
        nc.sync.dma_start(out=oneh_sb, in_=oneh_d)
        gidx_sb = singles.tile([128, NPASS * NQ], DT.int32, name="gidxsb")
        nc.sync.dma_start(out=gidx_sb, in_=gidx_d)
        ones_sb = singles.tile([BSH, 1], F32, name="onessb")
        nc.vector.memset(ones_sb, 1.0)

        # h_root[100, 0, :] is a ones-row so the classifier bias rides the matmul
        h_root = state_p.tile([128, KT, BSH], BF16, name="hroot", tag="hroot")
        nc.vector.memset(h_root[96:128, 0, :], 1.0)  # rows 96:100 later overwritten by h

        for g in range(NPASS):
            qbase = 0
            ch_in = cf_in = None
            for lvl in range(DEPTH, -1, -1):
                nl = 2 ** lvl
                rows = nl * G
                nq_lvl = nl
                if lvl > 0:
                    par = "A" if lvl % 2 == 0 else "B"
                    prow = (nl // 2) * G
                    ch_out = state_p.tile([TS, KT, prow], BF16, name=f"ch_{g}_{lvl}", tag="ch" + par)
                    cf_out = state_p.tile([TS, KT, prow], BF16, name=f"cf_{g}_{lvl}", tag="cf" + par)
                else:
                    ch_out = cf_out = None
                nchunk = max(rows // 512, 1)
                qpc = nq_lvl // nchunk
                N = min(rows, 512)
                for c in range(nchunk):
                    q0 = g * NQ + qbase + c * qpc
                    graw = graw_p.tile([128, 4, DIN], F32, name="graw", tag="graw")
                    for qi in range(qpc):
                        # one gather per quad: [128,1] index AP is the only
                        # form the HW descriptor unroll handles correctly
                        nc.gpsimd.indirect_dma_start(
                            out=graw[:, qi, :], out_offset=None,
                            in_=emb_d,
                            in_offset=IndirectOffsetOnAxis(
                                ap=gidx_sb[:, q0 + qi:q0 + qi + 1], axis=0),
                        )
                    xt = xt_p.tile([128, KT, 512], BF16, name="xt", tag="xt")
                    nc.vector.memset(xt[96:128, 0, :], 1.0)  # ones-row at 100; 96:100 overwritten by cast
                    for qi in range(qpc):
                        pst = ps_t.tile([TS, KT * 128], F32, name="pst", tag="pst")
                        for k in range(KT):
                            nc.tensor.transpose(
                                out=pst[:, k * 128:(k + 1) * 128],
                                in_=graw[:, qi, k * TS:(k + 1) * TS],
                                identity=ident)
                        # single fp32->bf16 cast per quad (3 K-tiles at once)
                        nc.vector.tensor_copy(
                            out=xt[:TS, :, qi * 128: (qi + 1) * 128],
                            in_=pst.rearrange("p (k c) -> p k c", k=KT)[:, :, :])
                    c0 = c * 512
                    gate_sb = {}
                    for gi in range(4):
                        if lvl == 0 and gi == 1:
                            continue
                        ps = ps_g.tile([TS, KT, 512], F32, name="psgate", tag="psg")
                        has_h = (lvl < DEPTH) and (gi in H_COL)
                        for m in range(KT):
                            for k in range(KT):
                                ksz = TS + 1 if k == 0 else TS  # ones/bias row
                                nc.tensor.matmul(
                                    out=ps[:, m, :N],
                                    lhsT=w4_sb[:ksz, k, gi * 300 + m * TS: gi * 300 + (m + 1) * TS],
                                    rhs=xt[:ksz, k, :N],
                                    start=(k == 0), stop=(k == KT - 1 and not has_h))
                            if has_h:
                                for k in range(KT):
                                    nc.tensor.matmul(
                                        out=ps[:, m, :N],
                                        lhsT=wh_sb[:TS, k, H_COL[gi] + m * TS: H_COL[gi] + (m + 1) * TS],
                                        rhs=ch_in[:, k, c0:c0 + N],
                                        start=False, stop=(k == KT - 1))
                        gsb = gate_p.tile([TS, KT, 512], F32, name=f"g{gi}", tag=GTAG[gi])
                        nc.scalar.activation(
                            out=gsb[:, :, :N], in_=ps[:, :, :N], func=FUNC[gi])
                        gate_sb[gi] = gsb
                    c_sb = gate_p.tile([TS, KT, 512], F32, name="csb", tag="g3")
                    nc.vector.tensor_mul(c_sb[:, :, :N], gate_sb[0][:, :, :N], gate_sb[3][:, :, :N])
                    if lvl < DEPTH:
                        nc.vector.tensor_add(c_sb[:, :, :N], c_sb[:, :, :N], cf_in[:, :, c0:c0 + N])
                    tc_sb = gate_p.tile([TS, KT, 512], F32, name="tcsb", tag="g0")
                    nc.scalar.activation(out=tc_sb[:, :, :N], in_=c_sb[:, :, :N], func=AF.Tanh)
                    if lvl > 0:
                        h_sb = gate_p.tile([TS, KT, 512], F32, name="hsb", tag="g2")
                        nc.vector.tensor_mul(h_sb[:, :, :N], gate_sb[2][:, :, :N], tc_sb[:, :, :N])
                        fc_sb = gate_p.tile([TS, KT, 512], F32, name="fcsb", tag="g1")
                        nc.vector.tensor_mul(fc_sb[:, :, :N], gate_sb[1][:, :, :N], c_sb[:, :, :N])
                        hv = h_sb[:, :, :N].rearrange("p k (nb two g) -> p k nb two g", two=2, g=G)
                        fv = fc_sb[:, :, :N].rearrange("p k (nb two g) -> p k nb two g", two=2, g=G)
                        po = c0 // 2
                        cho = ch_out[:, :, po:po + N // 2].rearrange("p k (nb g) -> p k nb g", g=G)
                        cfo = cf_out[:, :, po:po + N // 2].rearrange("p k (nb g) -> p k nb g", g=G)
                        nc.vector.tensor_add(cho, hv[:, :, :, 0, :], hv[:, :, :, 1, :])
                        nc.vector.tensor_add(cfo, fv[:, :, :, 0, :], fv[:, :, :, 1, :])
                    else:
                        nc.vector.tensor_mul(
                            h_root[:TS, :, g * G:(g + 1) * G],
                            gate_sb[2][:, :, :G], tc_sb[:, :, :G])
                qbase += nq_lvl
                ch_in, cf_in = ch_out, cf_out

        # classifier + log_softmax + sum(logp[gold]) over all 128 trees
        ps_cls = ps_g.tile([L, BSH], F32, name="pscls", tag="psg")
        for k in range(KT):
            ksz = TS + 1 if k == 0 else TS
            nc.tensor.matmul(out=ps_cls, lhsT=wout_sb[:ksz, k, :],
                             rhs=h_root[:ksz, k, :],
                             start=(k == 0), stop=(k == KT - 1))
        logitsT = gate_p.tile([L, BSH], F32, name="logitsT", tag="mA")
        nc.vector.tensor_copy(logitsT, ps_cls)
        ps_lg = ps_g.tile([BSH, L], F32, name="pslg", tag="psg")
        nc.tensor.transpose(out=ps_lg, in_=logitsT, identity=ident5)
        logits = gate_p.tile([BSH, L], F32, name="logits", tag="mB")
        nc.vector.tensor_copy(logits, ps_lg)
        msb = gate_p.tile([BSH, 1], F32, name="msb", tag="m1")
        nc.vector.reduce_max(out=msb, in_=logits, axis=mybir.AxisListType.X)
        negm = gate_p.tile([BSH, 1], F32, name="negm", tag="m2")
        nc.vector.tensor_scalar_mul(negm, msb, -1.0)
        esb = gate_p.tile([BSH, L], F32, name="esb", tag="mC")
        nc.scalar.activation(out=esb, in_=logits, func=AF.Exp, bias=negm[:, 0:1])
        ssb = gate_p.tile([BSH, 1], F32, name="ssb", tag="m3")
        nc.vector.reduce_sum(out=ssb, in_=esb, axis=mybir.AxisListType.X)
        lssb = gate_p.tile([BSH, 1], F32, name="lssb", tag="m4")
        nc.scalar.activation(out=lssb, in_=ssb, func=AF.Ln)
        tot = gate_p.tile([BSH, 1], F32, name="tot", tag="m5")
        nc.vector.tensor_add(tot, msb, lssb)
        logp_sb = gate_p.tile([BSH, L], F32, name="logpsb", tag="mD")
        nc.vector.tensor_scalar_sub(logp_sb, logits, tot[:, 0:1])
        nc.sync.dma_start(out=logp_d, in_=logp_sb)
        prod = gate_p.tile([BSH, L], F32, name="prod", tag="mE")
        nc.vector.tensor_mul(prod, logp_sb, oneh_sb)
        ptree = gate_p.tile([BSH, 1], F32, name="ptree", tag="m6")
        nc.vector.reduce_sum(out=ptree, in_=prod, axis=mybir.AxisListType.X)
        ps_nll = ps_g.tile([1, 1], F32, name="psnll", tag="psg")
        nc.tensor.matmul(out=ps_nll, lhsT=ptree, rhs=ones_sb, start=True, stop=True)
        nll_sb = gate_p.tile([1, 1], F32, name="nllsb", tag="m7")
        nc.vector.tensor_copy(nll_sb, ps_nll)
        nc.sync.dma_start(out=nll_d, in_=nll_sb)
    nc.finalize()
    return nc


def _pad_ktiles(w, cols, bias=None):
    """[300, cols] fp32 -> [KT, 128, cols] bf16; bias goes in row 100 of K-tile 0."""
    out = np.zeros((KT, 128, cols), dtype=ml_dtypes.bfloat16)
    for k in range(KT):
        out[k, :TS, :] = w[k * TS:(k + 1) * TS, :].astype(ml_dtypes.bfloat16)
    if bias is not None:
        out[0, TS, :] = bias.astype(ml_dtypes.bfloat16)
    return out


def prep_in_maps(inputs):
    wi = np.asarray(inputs["word_idx"]).astype(np.int32)
    gold = np.asarray(inputs["gold"]).astype(np.int64)
    emb = np.ascontiguousarray(np.asarray(inputs["embedding"], dtype=np.float32))
    W4 = np.concatenate(
        [np.asarray(inputs["W_" + n], dtype=np.float32) for n in ("ix", "fx", "ox", "ux")],
        axis=1)
    Wh = np.concatenate(
        [np.asarray(inputs["W_" + n], dtype=np.float32) for n in ("ih", "oh", "uh")],
        axis=1)
    b4 = np.concatenate([
        np.asarray(inputs["b_ix"]) + np.asarray(inputs["b_ih"]),
        np.asarray(inputs["b_fx"]) + np.asarray(inputs["b_fh"]),
        np.asarray(inputs["b_ox"]) + np.asarray(inputs["b_oh"]),
        np.asarray(inputs["b_ux"]) + np.asarray(inputs["b_uh"]),
    ]).astype(np.float32)
    wout = np.asarray(inputs["W_out"], dtype=np.float32)
    bout = np.asarray(inputs["b_out"], dtype=np.float32)
    norder = np.array(node_list())
    eye = np.eye(L, dtype=np.float32)

    w4_p = _pad_ktiles(W4, 1200, bias=b4)
    wh_p = _pad_ktiles(Wh, 900)
    wout_p = _pad_ktiles(wout, L, bias=bout)

    in_maps = []
    for c in range(NCORES):
        t0 = c * BSH
        gidx = np.ascontiguousarray(wi[t0:t0 + BSH][:, norder])  # [128, 127]
        in_maps.append(dict(
            gidx=gidx, emb=emb, w4=w4_p, wh=wh_p,
            wout=wout_p,
            onehot=np.ascontiguousarray(eye[gold[t0:t0 + BSH]])))
    return in_maps


_PROG = None


def _get_prog():
    global _PROG
    if _PROG is None:
        _PROG = build_program()
    return _PROG


def _assemble(results):
    logp = np.concatenate([results[c]["logp"] for c in range(NCORES)], axis=0)
    tot = sum(float(results[c]["nll"][0, 0]) for c in range(NCORES))
    loss = np.float32(-tot / B)
    return np.ascontiguousarray(logp.astype(np.float32)), loss


def kernel(**inputs):
    nc = _get_prog()
    in_maps = prep_in_maps(inputs)
    res = run_bass_kernel_spmd(nc, in_maps, list(range(NCORES)))
    return _assemble(res.results)


def kernel_profiled(**inputs):
    """Same as kernel() but with NTFF tracing; returns (outputs, exec_time_ns)."""
    nc = _get_prog()
    in_maps = prep_in_maps(inputs)
    res = run_bass_kernel_spmd(nc, in_maps, list(range(NCORES)), trace=True)
    return _assemble(res.results), res.exec_time_ns
